# revision 20
# baseline (speedup 1.0000x reference)
"""Conv2D 3x3 stride-1 pad-1 (NCHW) as implicit GEMM on 8 NeuronCores.

Strategy: data-parallel over batch (32 imgs -> 4 per core). The input is
zero-padded on the host to (*, 128, 58, 58) and cast to fp16 so each image
DMAs into an SBUF tile [C=128, 58, 58] with input channels on partitions.
Weights are preprocessed host-side to [I=128, oc_chunk, (kh kw) * 128] fp16
so each (oc, tap) slice is a ready [K=128, M=128] stationary operand.

Loop order is taps-INNER: for each output row-group (8 rows, free dim
8*56=448) the 9 filter taps accumulate back-to-back into one PSUM bank,
so finished groups stream out (bias-add on DVE + DMA) continuously
instead of bursting at the end of a pass. fp16 operands keep the per-MM
implicit LDWEIGHTS (~97ns) hidden under the 448-cycle matmul streaming;
fp32 weights would serialize at ~223ns/LDW and pace the whole kernel.

x (4,128,58,58) fp16 -> out (4,256,56,56) f32 per core; no collectives.
"""

import os
import sys

import numpy as np

if "/opt/trn_rl_repo" not in sys.path:
    sys.path.insert(0, "/opt/trn_rl_repo")

from concourse import bacc, bass, mybir  # noqa: E402
from concourse.bass_utils import run_bass_kernel_spmd  # noqa: E402
from concourse.tile import TileContext, add_dep_helper  # noqa: E402

N_FULL, CIN, H, W = 32, 128, 56, 56
COUT = 256
KH = KW = 3
NCORES = 8
NPER = N_FULL // NCORES  # 4 images per core
HP, WP = H + 2, W + 2  # 58 x 58 padded
ROWS = 8  # output rows per matmul group
NFREE = ROWS * W  # 448 moving free dim (<= 512 PSUM bank limit)
NGROUPS = H // ROWS  # 7
OCH = COUT // 128  # 2 output-channel chunks
NTAPS = KH * KW  # 9
NXBUF = 3  # x image buffers (triple buffer so loads run a full pass early)

MODE = os.environ.get("CONV_MM_MODE", "fp16")

_CACHE = {}


def _build_conv(mode):
    f32 = mybir.dt.float32
    io_dt = {
        "fp16": mybir.dt.float16,
        "bf16": mybir.dt.bfloat16,
        "fp32": f32,
        "fp32r": f32,
    }[mode]
    mm_dt = mybir.dt.float32r if mode == "fp32r" else io_dt

    # Bacc (not raw Bass): its compile pipeline legalizes sync waits --
    # TRN2 instructions carry at most one wait slot.
    nc = bacc.Bacc(None, target_bir_lowering=False)

    x_par = nc.declare_dram_parameter("x", [NPER, CIN, HP, WP], io_dt, isOutput=False)
    w_par = nc.declare_dram_parameter(
        "wt", [CIN, OCH, NTAPS * 128], io_dt, isOutput=False
    )
    bias_par = nc.declare_dram_parameter("bias", [COUT], f32, isOutput=False)
    out_par = nc.declare_dram_parameter("out", [NPER, COUT, H, W], f32, isOutput=True)
    out_flat = out_par.rearrange("n o h w -> n o (h w)")

    def mmv(ap):
        return ap.bitcast(mm_dt) if mm_dt != io_dt else ap

    with TileContext(nc) as tc:
        with (
            tc.tile_pool(name="const", bufs=1) as cpool,
            tc.tile_pool(name="xpad", bufs=1) as xpool,
            tc.tile_pool(name="psum", bufs=8, space="PSUM") as ppool,
            tc.tile_pool(name="outp", bufs=6) as opool,
        ):
            # HAM pre-warm: short junk matmuls gated only on a prologue
            # memset (vector engine -- its preamble clears earliest) keep
            # the PE busy through the initial DMA wait so the clock gate is
            # released (2.4 GHz) near the start of the real stream. N=128
            # keeps them fine-grained so real matmuls slot in promptly.
            jnk = cpool.tile([128, 128], mm_dt, tag="jnk")
            nc.vector.memset(jnk[:], 1.0)
            ps_jnk = ppool.tile([128, NFREE], f32, tag="ps", name="ps")
            for _ in range(20):
                nc.tensor.matmul(
                    ps_jnk[:, 0:128], jnk[:], jnk[:], start=True, stop=True
                )

            # x image buffers (zero borders come in with the host-padded DMA)
            xpads = [
                xpool.tile([CIN, HP, WP], mm_dt, tag=f"xpad{b}", name="xpad")
                for b in range(NXBUF)
            ]
            # weights: one tile per oc chunk, [CIN, (tap m)]
            w_sb = [
                cpool.tile([CIN, NTAPS * 128], mm_dt, tag=f"w{oc}", name="w")
                for oc in range(OCH)
            ]
            bias_sb = cpool.tile([128, OCH], f32, tag="bias")

            # Head loads. Constraints: a single dma_start tops out ~155 GB/s
            # and each issue costs ~0.6-0.7us serially on its queue's
            # sequencer, so the critical first tensors are spread across the
            # sync / scalar / gpsimd queues and chunked so group-0 compute
            # can start while later rows are still in flight.
            # Head queues ramp slowly (~60-100 GB/s for their first transfers,
            # 200+ GB/s later), so the two items the first matmul group needs
            # -- x rows 0-7 and w tap0 -- lead the two earliest queues:
            # sync:   x rows 0-8 | x 10-26 | x 26-42 | x 42-58
            # scalar: w tap0 | w taps 1-2 | x rows 8-10 | bias | odd outputs
            # gpsimd: w taps 3-5 | w taps 6-8 | w oc1 | images
            nc.sync.dma_start(out=xpads[0][:, 0:8, :], in_=mmv(x_par[0])[:, 0:8, :])
            nc.scalar.dma_start(out=w_sb[0][:, 0:128], in_=mmv(w_par)[:, 0, 0:128])
            nc.gpsimd.dma_start(
                out=w_sb[0][:, 3 * 128 : 6 * 128],
                in_=mmv(w_par)[:, 0, 3 * 128 : 6 * 128],
            )
            nc.scalar.dma_start(
                out=w_sb[0][:, 128 : 3 * 128], in_=mmv(w_par)[:, 0, 128 : 3 * 128]
            )
            nc.gpsimd.dma_start(
                out=w_sb[0][:, 6 * 128 :], in_=mmv(w_par)[:, 0, 6 * 128 :]
            )
            nc.scalar.dma_start(
                out=xpads[0][:, 8:10, :], in_=mmv(x_par[0])[:, 8:10, :]
            )
            for r0, r1 in [(10, 26), (26, 42), (42, 58)]:
                nc.sync.dma_start(
                    out=xpads[0][:, r0:r1, :], in_=mmv(x_par[0])[:, r0:r1, :]
                )
            nc.scalar.dma_start(
                out=bias_sb[:], in_=bias_par.rearrange("(a b) -> b a", b=128)
            )
            nc.gpsimd.dma_start(out=w_sb[1][:], in_=mmv(w_par)[:, 1, :])

            # Image 1/2 land in fresh buffers (no WAR dep), so they are gated
            # behind the first real matmul to keep the head HBM window clean.
            # Image 3 reuses buffer 0: its dma_start MUST be emitted after
            # pass 0's matmuls in program order (inside the n loop below) so
            # the tile framework orders it write-after-read of pass 0 -- and
            # pass 0 reads image 0, not image 3.
            img_dmas = [
                nc.gpsimd.dma_start(out=xpads[n % NXBUF][:], in_=mmv(x_par[n]))
                for n in range(1, NXBUF)
            ]

            mm_first = None
            out_q = [nc.sync, nc.scalar]  # alternate output DMA queues
            qi = 0
            for n in range(NPER):
                xt = xpads[n % NXBUF]
                # load image n+2 into the buffer pass n-1 just released
                if NXBUF <= n + 2 < NPER:
                    nc.gpsimd.dma_start(
                        out=xpads[(n + 2) % NXBUF][:], in_=mmv(x_par[n + 2])
                    )
                for oc in range(OCH):
                    for g in range(NGROUPS):
                        ps = ppool.tile([128, NFREE], f32, tag="ps", name="ps")
                        for tap in range(NTAPS):
                            kh, kw = divmod(tap, KW)
                            mm = nc.tensor.matmul(
                                ps[:],
                                w_sb[oc][:, tap * 128 : (tap + 1) * 128],
                                xt[:, g * ROWS + kh : g * ROWS + kh + ROWS, kw : kw + W],
                                start=(tap == 0),
                                stop=(tap == NTAPS - 1),
                            )
                            if mm_first is None:
                                mm_first = mm
                        ot = opool.tile([128, NFREE], f32, tag="ot", name="ot")
                        dst = out_flat[
                            n, oc * 128 : (oc + 1) * 128, g * NFREE : (g + 1) * NFREE
                        ]
                        last = n == NPER - 1 and oc == OCH - 1 and g == NGROUPS - 1
                        if last:
                            # split the final bias-add + DMA into halves on
                            # both queues so they pipeline off the last matmul
                            hf = NFREE // 2
                            for h in range(2):
                                sl = slice(h * hf, (h + 1) * hf)
                                nc.vector.tensor_scalar_add(
                                    out=ot[:, sl],
                                    in0=ps[:, sl],
                                    scalar1=bias_sb[:, oc : oc + 1],
                                )
                                out_q[h].dma_start(out=dst[:, sl], in_=ot[:, sl])
                        else:
                            nc.vector.tensor_scalar_add(
                                out=ot[:], in0=ps[:], scalar1=bias_sb[:, oc : oc + 1]
                            )
                            out_q[qi].dma_start(out=dst, in_=ot[:])
                            qi ^= 1
            for d in img_dmas:
                add_dep_helper(
                    d.ins, mm_first.ins, sync=True, reason="defer past first matmul"
                )
    nc.compile()
    return nc


def _get_nc(mode):
    if mode not in _CACHE:
        _CACHE[mode] = _build_conv(mode)
    return _CACHE[mode]


# test-harness hooks: set TRACE=True before calling kernel() to capture an
# NTFF profile; LAST_RESULTS then holds the BassKernelResults.
TRACE = False
LAST_RESULTS = None


def kernel(x, weight, bias):
    global LAST_RESULTS
    mode = MODE
    x = np.ascontiguousarray(np.asarray(x), dtype=np.float32)
    w = np.ascontiguousarray(np.asarray(weight), dtype=np.float32)
    b = np.ascontiguousarray(np.asarray(bias), dtype=np.float32)
    xp = np.pad(x, ((0, 0), (0, 0), (1, 1), (1, 1)))
    # wt[i, oc, (kh kw m)] = w[oc*128 + m, i, kh, kw]
    wt = np.ascontiguousarray(
        w.transpose(1, 2, 3, 0)
        .reshape(CIN, NTAPS, OCH, 128)
        .transpose(0, 2, 1, 3)
        .reshape(CIN, OCH, NTAPS * 128)
    )

    if mode in ("fp32", "fp32r"):
        xc, wc = xp, wt
    elif mode == "fp16":
        xc, wc = xp.astype(np.float16), wt.astype(np.float16)
    else:  # bf16
        import ml_dtypes

        xc = xp.astype(ml_dtypes.bfloat16)
        wc = wt.astype(ml_dtypes.bfloat16)

    per_core = [
        {"x": xc[c * NPER : (c + 1) * NPER], "wt": wc, "bias": b}
        for c in range(NCORES)
    ]

    kwargs = {}
    if TRACE:
        kwargs = dict(trace=True, trace_cores=[0])
    res = run_bass_kernel_spmd(
        _get_nc(mode), per_core, core_ids=list(range(NCORES)), **kwargs
    )
    LAST_RESULTS = res
    return np.concatenate([r["out"] for r in res.results], axis=0)


# revision 21
# speedup vs baseline: 1.0216x; 1.0216x over previous
"""Conv2D 3x3 stride-1 pad-1 (NCHW) as implicit GEMM on 8 NeuronCores.

Strategy: data-parallel over batch (32 imgs -> 4 per core). The input is
zero-padded on the host to (*, 128, 58, 58) and cast to fp16 so each image
DMAs into an SBUF tile [C=128, 58, 58] with input channels on partitions.
Weights are preprocessed host-side to [I=128, oc_chunk, (kh kw) * 128] fp16
so each (oc, tap) slice is a ready [K=128, M=128] stationary operand.

Loop order is taps-INNER: for each output row-group (8 rows, free dim
8*56=448) the 9 filter taps accumulate back-to-back into one PSUM bank,
so finished groups stream out (bias-add on DVE + DMA) continuously
instead of bursting at the end of a pass. fp16 operands keep the per-MM
implicit LDWEIGHTS (~97ns) hidden under the 448-cycle matmul streaming;
fp32 weights would serialize at ~223ns/LDW and pace the whole kernel.

x (4,128,58,58) fp16 -> out (4,256,56,56) f32 per core; no collectives.
"""

import os
import sys

import numpy as np

if "/opt/trn_rl_repo" not in sys.path:
    sys.path.insert(0, "/opt/trn_rl_repo")

from concourse import bacc, bass, mybir  # noqa: E402
from concourse.bass_utils import run_bass_kernel_spmd  # noqa: E402
from concourse.tile import TileContext, add_dep_helper  # noqa: E402

N_FULL, CIN, H, W = 32, 128, 56, 56
COUT = 256
KH = KW = 3
NCORES = 8
NPER = N_FULL // NCORES  # 4 images per core
HP, WP = H + 2, W + 2  # 58 x 58 padded
ROWS = 8  # output rows per matmul group
NFREE = ROWS * W  # 448 moving free dim (<= 512 PSUM bank limit)
NGROUPS = H // ROWS  # 7
OCH = COUT // 128  # 2 output-channel chunks
NTAPS = KH * KW  # 9
NXBUF = 3  # x image buffers (triple buffer so loads run a full pass early)

MODE = os.environ.get("CONV_MM_MODE", "fp16")

_CACHE = {}


def _build_conv(mode):
    f32 = mybir.dt.float32
    io_dt = {
        "fp16": mybir.dt.float16,
        "bf16": mybir.dt.bfloat16,
        "fp32": f32,
        "fp32r": f32,
    }[mode]
    mm_dt = mybir.dt.float32r if mode == "fp32r" else io_dt

    # Bacc (not raw Bass): its compile pipeline legalizes sync waits --
    # TRN2 instructions carry at most one wait slot.
    nc = bacc.Bacc(None, target_bir_lowering=False)

    x_par = nc.declare_dram_parameter("x", [NPER, CIN, HP, WP], io_dt, isOutput=False)
    w_par = nc.declare_dram_parameter(
        "wt", [CIN, OCH, NTAPS * 128], io_dt, isOutput=False
    )
    bias_par = nc.declare_dram_parameter("bias", [COUT], f32, isOutput=False)
    out_par = nc.declare_dram_parameter("out", [NPER, COUT, H, W], f32, isOutput=True)
    out_flat = out_par.rearrange("n o h w -> n o (h w)")

    def mmv(ap):
        return ap.bitcast(mm_dt) if mm_dt != io_dt else ap

    with TileContext(nc) as tc:
        with (
            tc.tile_pool(name="const", bufs=1) as cpool,
            tc.tile_pool(name="xpad", bufs=1) as xpool,
            tc.tile_pool(name="psum", bufs=8, space="PSUM") as ppool,
            tc.tile_pool(name="outp", bufs=6) as opool,
        ):
            # HAM pre-warm: short junk matmuls gated only on a prologue
            # memset (vector engine -- its preamble clears earliest) keep
            # the PE busy through the initial DMA wait so the clock gate is
            # released (2.4 GHz) near the start of the real stream. N=128
            # keeps them fine-grained so real matmuls slot in promptly.
            jnk = cpool.tile([128, 128], mm_dt, tag="jnk")
            nc.vector.memset(jnk[:], 1.0)
            ps_jnk = ppool.tile([128, NFREE], f32, tag="ps", name="ps")
            for _ in range(20):
                nc.tensor.matmul(
                    ps_jnk[:, 0:128], jnk[:], jnk[:], start=True, stop=True
                )

            # x image buffers (zero borders come in with the host-padded DMA)
            xpads = [
                xpool.tile([CIN, HP, WP], mm_dt, tag=f"xpad{b}", name="xpad")
                for b in range(NXBUF)
            ]
            # weights: one tile per oc chunk, [CIN, (tap m)]
            w_sb = [
                cpool.tile([CIN, NTAPS * 128], mm_dt, tag=f"w{oc}", name="w")
                for oc in range(OCH)
            ]
            bias_sb = cpool.tile([128, OCH], f32, tag="bias")

            # Head loads. Constraints: a single dma_start tops out ~155 GB/s
            # and each issue costs ~0.6-0.7us serially on its queue's
            # sequencer, so the critical first tensors are spread across the
            # sync / scalar / gpsimd queues and chunked so group-0 compute
            # can start while later rows are still in flight.
            # Head queues ramp slowly (~60-100 GB/s for their first transfers,
            # 200+ GB/s later), so the two items the first matmul group needs
            # -- x rows 0-7 and w tap0 -- lead the two earliest queues:
            # sync:   x rows 0-8 | x 10-26 | x 26-42 | x 42-58
            # scalar: w tap0 | w taps 1-2 | x rows 8-10 | bias | odd outputs
            # gpsimd: w taps 3-5 | w taps 6-8 | w oc1 | images
            nc.sync.dma_start(out=xpads[0][:, 0:8, :], in_=mmv(x_par[0])[:, 0:8, :])
            nc.scalar.dma_start(
                out=w_sb[0][:, 0 : 3 * 128], in_=mmv(w_par)[:, 0, 0 : 3 * 128]
            )
            nc.gpsimd.dma_start(
                out=w_sb[0][:, 3 * 128 : 6 * 128],
                in_=mmv(w_par)[:, 0, 3 * 128 : 6 * 128],
            )
            nc.gpsimd.dma_start(
                out=w_sb[0][:, 6 * 128 :], in_=mmv(w_par)[:, 0, 6 * 128 :]
            )
            nc.scalar.dma_start(
                out=xpads[0][:, 8:10, :], in_=mmv(x_par[0])[:, 8:10, :]
            )
            for r0, r1 in [(10, 26), (26, 42), (42, 58)]:
                nc.sync.dma_start(
                    out=xpads[0][:, r0:r1, :], in_=mmv(x_par[0])[:, r0:r1, :]
                )
            nc.scalar.dma_start(
                out=bias_sb[:], in_=bias_par.rearrange("(a b) -> b a", b=128)
            )
            nc.gpsimd.dma_start(out=w_sb[1][:], in_=mmv(w_par)[:, 1, :])

            # Image 1/2 land in fresh buffers (no WAR dep), so they are gated
            # behind the first real matmul to keep the head HBM window clean.
            # Image 3 reuses buffer 0: its dma_start MUST be emitted after
            # pass 0's matmuls in program order (inside the n loop below) so
            # the tile framework orders it write-after-read of pass 0 -- and
            # pass 0 reads image 0, not image 3.
            img_dmas = [
                nc.gpsimd.dma_start(out=xpads[n % NXBUF][:], in_=mmv(x_par[n]))
                for n in range(1, NXBUF)
            ]

            mm_first = None
            out_q = [nc.sync, nc.scalar]  # alternate output DMA queues
            qi = 0
            for n in range(NPER):
                xt = xpads[n % NXBUF]
                # load image n+2 into the buffer pass n-1 just released
                if NXBUF <= n + 2 < NPER:
                    nc.gpsimd.dma_start(
                        out=xpads[(n + 2) % NXBUF][:], in_=mmv(x_par[n + 2])
                    )
                for oc in range(OCH):
                    for g in range(NGROUPS):
                        ps = ppool.tile([128, NFREE], f32, tag="ps", name="ps")
                        for tap in range(NTAPS):
                            kh, kw = divmod(tap, KW)
                            mm = nc.tensor.matmul(
                                ps[:],
                                w_sb[oc][:, tap * 128 : (tap + 1) * 128],
                                xt[:, g * ROWS + kh : g * ROWS + kh + ROWS, kw : kw + W],
                                start=(tap == 0),
                                stop=(tap == NTAPS - 1),
                            )
                            if mm_first is None:
                                mm_first = mm
                        ot = opool.tile([128, NFREE], f32, tag="ot", name="ot")
                        dst = out_flat[
                            n, oc * 128 : (oc + 1) * 128, g * NFREE : (g + 1) * NFREE
                        ]
                        last = n == NPER - 1 and oc == OCH - 1 and g == NGROUPS - 1
                        if last:
                            # split the final bias-add + DMA into halves on
                            # both queues so they pipeline off the last matmul
                            hf = NFREE // 2
                            for h in range(2):
                                sl = slice(h * hf, (h + 1) * hf)
                                nc.vector.tensor_scalar_add(
                                    out=ot[:, sl],
                                    in0=ps[:, sl],
                                    scalar1=bias_sb[:, oc : oc + 1],
                                )
                                out_q[h].dma_start(out=dst[:, sl], in_=ot[:, sl])
                        else:
                            nc.vector.tensor_scalar_add(
                                out=ot[:], in0=ps[:], scalar1=bias_sb[:, oc : oc + 1]
                            )
                            out_q[qi].dma_start(out=dst, in_=ot[:])
                            qi ^= 1
            for d in img_dmas:
                add_dep_helper(
                    d.ins, mm_first.ins, sync=True, reason="defer past first matmul"
                )
    nc.compile()
    return nc


def _get_nc(mode):
    if mode not in _CACHE:
        _CACHE[mode] = _build_conv(mode)
    return _CACHE[mode]


# test-harness hooks: set TRACE=True before calling kernel() to capture an
# NTFF profile; LAST_RESULTS then holds the BassKernelResults.
TRACE = False
LAST_RESULTS = None


def kernel(x, weight, bias):
    global LAST_RESULTS
    mode = MODE
    x = np.ascontiguousarray(np.asarray(x), dtype=np.float32)
    w = np.ascontiguousarray(np.asarray(weight), dtype=np.float32)
    b = np.ascontiguousarray(np.asarray(bias), dtype=np.float32)
    xp = np.pad(x, ((0, 0), (0, 0), (1, 1), (1, 1)))
    # wt[i, oc, (kh kw m)] = w[oc*128 + m, i, kh, kw]
    wt = np.ascontiguousarray(
        w.transpose(1, 2, 3, 0)
        .reshape(CIN, NTAPS, OCH, 128)
        .transpose(0, 2, 1, 3)
        .reshape(CIN, OCH, NTAPS * 128)
    )

    if mode in ("fp32", "fp32r"):
        xc, wc = xp, wt
    elif mode == "fp16":
        xc, wc = xp.astype(np.float16), wt.astype(np.float16)
    else:  # bf16
        import ml_dtypes

        xc = xp.astype(ml_dtypes.bfloat16)
        wc = wt.astype(ml_dtypes.bfloat16)

    per_core = [
        {"x": xc[c * NPER : (c + 1) * NPER], "wt": wc, "bias": b}
        for c in range(NCORES)
    ]

    kwargs = {}
    if TRACE:
        kwargs = dict(trace=True, trace_cores=[0])
    res = run_bass_kernel_spmd(
        _get_nc(mode), per_core, core_ids=list(range(NCORES)), **kwargs
    )
    LAST_RESULTS = res
    return np.concatenate([r["out"] for r in res.results], axis=0)
